# revision 27
# baseline (speedup 1.0000x reference)
"""Axial (per-row) pair attention kernel for Trainium2, 8-core SPMD.

Contract: kernel(**inputs) takes the FULL unsharded inputs from
setup_inputs() and returns the FULL (2,128,128,256) float32 output.

Sharding: the (b, s1) row axis (2*128 = 256 independent attention rows) is
split evenly across 8 NeuronCores; each core runs the identical Bass program
on its 32-row slice.

v2 design (vs the repack-based baseline):
 - Scores run as row-tiled matmuls (tile_position via base-partition
   slices): head g of chunk ec lives at partitions 32g of the natural
   QKV e-chunk layout, so the (32, head, tok) repack is gone entirely.
 - rotate_half(q) is folded into the projection weights on the host
   (W_rot = W[:, :32] @ R^T), so q_rot/k_rot fall out of the same QKV
   matmul at partitions 0-31 and rotary is 3 wide DVE ops.
 - One exp activation per row ([128, 8*128], mask as per-partition bias).
 - Copy work split across DVE (qk/v/normalize/transposeouts) and ACT
   (exp, y writeback) to balance engine busy time.
"""

import numpy as np

import concourse.bass as bass
import concourse.mybir as mybir
import concourse.tile as tile
from concourse import bacc
from concourse.bass_utils import run_bass_kernel_spmd
from concourse.masks import make_identity

N_CORES = 8
B, S, D = 2, 128, 256
H, HD, ROT = 8, 32, 32
NROWS = B * S
RPC = NROWS // N_CORES  # rows per core = 32
SCALE = HD ** -0.5
LN_EPS = 1e-5
MASK_BIAS = -1e9
EW = 3 * D + 2 * ROT  # 832 projection channels (q|k|v|qrot|krot)

F32 = mybir.dt.float32
F16 = mybir.dt.float16  # matmul-input dtype (fp32 accumulate in PSUM)


def _build_bass() -> bass.Bass:
    nc = bacc.Bacc(None)

    x = nc.dram_tensor("x", [RPC, S, D], F32, kind="ExternalInput")
    cos_t = nc.dram_tensor("cos_t", [ROT, RPC, S], F16, kind="ExternalInput")
    sin_t = nc.dram_tensor("sin_t", [ROT, RPC, S], F16, kind="ExternalInput")
    maskb = nc.dram_tensor("maskb", [S, RPC], F32, kind="ExternalInput")
    wqkv = nc.dram_tensor("wqkv", [2, 128, EW], F16, kind="ExternalInput")
    wout = nc.dram_tensor("wout", [2, 128, D], F16, kind="ExternalInput")
    y = nc.dram_tensor("y", [RPC, S, D], F32, kind="ExternalOutput")

    with tile.TileContext(nc) as tc:
        with (
            tc.tile_pool(name="consts", bufs=1) as consts,
            tc.tile_pool(name="xpool", bufs=RPC) as xpool,
            tc.tile_pool(name="lnpool", bufs=4) as lnpool,
            tc.tile_pool(name="tpool", bufs=2) as tpool,
            tc.tile_pool(name="qkpool", bufs=2) as qkpool,
            tc.tile_pool(name="vpool", bufs=2) as vpool,
            tc.tile_pool(name="epool", bufs=2) as epool,
            tc.tile_pool(name="apool", bufs=4) as apool,
            tc.tile_pool(name="ypool", bufs=2) as ypool,
            tc.tile_pool(name="ps_trot", bufs=1, space="PSUM") as ps_trot,
            tc.tile_pool(name="ps_ring", bufs=2, space="PSUM") as ps_ring,
            tc.tile_pool(name="ps_v", bufs=1, space="PSUM") as ps_v,
            tc.tile_pool(name="ps_s", bufs=1, space="PSUM") as ps_s,
        ):
            # ---- constants ----
            ident = consts.tile([128, 128], F16)
            make_identity(nc, ident)
            wqkv_sb = consts.tile([128, 2, EW], F16)
            for c in range(2):
                nc.sync.dma_start(out=wqkv_sb[:, c, :], in_=wqkv[c])
            wout_sb = consts.tile([128, 2, D], F16)
            for c in range(2):
                nc.sync.dma_start(out=wout_sb[:, c, :], in_=wout[c])
            maskb_sb = consts.tile([S, RPC], F32)
            nc.sync.dma_start(out=maskb_sb, in_=maskb[:])
            eps_sb = consts.tile([128, 1], F32)
            nc.vector.memset(eps_sb, LN_EPS)
            cos_sb = consts.tile([ROT, RPC, S], F16)
            sin_sb = consts.tile([ROT, RPC, S], F16)
            nc.sync.dma_start(out=cos_sb, in_=cos_t[:])
            nc.sync.dma_start(out=sin_sb, in_=sin_t[:])

            # ---- prologue: loads + LN statistics for all rows ----
            mv_all = consts.tile([S, RPC, 2], F32)
            rstd_all = consts.tile([S, RPC], F32)
            x_tiles = []
            for p in range(RPC // 2):
                x_sb = xpool.tile([S, 2, D], F32)
                nc.sync.dma_start(
                    out=x_sb, in_=x[2 * p:2 * p + 2].rearrange("r t d -> t r d")
                )
                x_tiles.extend([x_sb[:, 0, :], x_sb[:, 1, :]])
                stats = lnpool.tile([S, 2, 6], F32, tag="stats")
                for j in range(2):
                    nc.vector.bn_stats(out=stats[:, j, :], in_=x_sb[:, j, :])
                    nc.vector.bn_aggr(
                        out=mv_all[:, 2 * p + j, :], in_=stats[:, j, :]
                    )
            # rstd for all rows in two wide instructions
            nc.scalar.activation(
                out=rstd_all, in_=mv_all[:, :, 1],
                func=mybir.ActivationFunctionType.Sqrt,
                bias=eps_sb, scale=1.0,
            )
            nc.vector.reciprocal(out=rstd_all, in_=rstd_all)

            def phase1(p):
                # LN apply, transpose, QKV+rot projection, rotary, v
                r0 = 2 * p
                xn_pair = lnpool.tile([S, 2, D], F16, tag="xn")
                for j in range(2):
                    nc.vector.tensor_scalar(
                        out=xn_pair[:, j, :], in0=x_tiles[r0 + j],
                        scalar1=mv_all[:, r0 + j, 0:1],
                        scalar2=rstd_all[:, r0 + j:r0 + j + 1],
                        op0=mybir.AluOpType.subtract, op1=mybir.AluOpType.mult,
                    )

                # ---- transpose xn -> (d-chunk, row, tok) ----
                t_alloc = ps_trot.tile([128, 2, 2, 2, S], F16, tag="trot")
                t_ps = t_alloc[:, 0, :, :, :]
                for j in range(2):
                    for c in range(2):
                        nc.tensor.transpose(
                            t_ps[:, c, j, :],
                            xn_pair[:, j, c * 128:(c + 1) * 128], ident,
                        )
                xnT_sb = tpool.tile([128, 2, 2, S], F16)
                nc.vector.tensor_copy(
                    out=xnT_sb.rearrange("p c j s -> p (c j s)"),
                    in_=t_ps.rearrange("p c j s -> p (c j s)"),
                )

                # ---- q/k projection: out (e-chunk, row, tok), through a
                # single shared PSUM bank (q then k), v/rot MMs fill the
                # PE while each half is copied out ----
                qk_sb = qkpool.tile([128, 2, 2, 2, S], F16, tag="qksb")
                rot_ps = ps_trot.tile([ROT, 2, 2, S], F32, tag="trot")
                v_ps = ps_v.tile([S, 2, D], F32, tag="vps")
                for qk in range(2):
                    half_ps = ps_ring.tile([128, 2, 2, S], F32, tag="ring")
                    for ec in range(2):
                        col = qk * D + ec * 128
                        for dc in range(2):
                            nc.tensor.matmul(
                                half_ps[:, ec, :, :],
                                lhsT=wqkv_sb[:, dc, col:col + 128],
                                rhs=xnT_sb[:, dc, :, :],
                                start=(dc == 0), stop=(dc == 1),
                            )
                    if qk == 0:
                        # rot/v matmuls keep the PE busy during the q copy
                        for rqk in range(2):
                            col = 3 * D + rqk * ROT
                            for dc in range(2):
                                nc.tensor.matmul(
                                    rot_ps[:, rqk, :, :],
                                    lhsT=wqkv_sb[:, dc, col:col + ROT],
                                    rhs=xnT_sb[:, dc, :, :],
                                    start=(dc == 0), stop=(dc == 1),
                                )
                    else:
                        for j in range(2):
                            for dc in range(2):
                                nc.tensor.matmul(
                                    v_ps[:, j, :],
                                    lhsT=xnT_sb[:, dc, j, :],
                                    rhs=wqkv_sb[:, dc, 2 * D:3 * D],
                                    start=(dc == 0), stop=(dc == 1),
                                )
                    # q half via DVE, k half via ACT (engine balance)
                    (nc.vector.tensor_copy if qk == 0 else nc.scalar.copy)(
                        out=qk_sb[:, qk].rearrange("p e j s -> p (e j s)"),
                        in_=half_ps.rearrange("p e j s -> p (e j s)"),
                    )

                # ---- rotary on head 0 (partitions 0-31 of ec 0) ----
                cs = cos_sb[:, r0:r0 + 2, :]
                sn = sin_sb[:, r0:r0 + 2, :]
                cos_b = bass.AP(
                    tensor=cos_sb.tensor, offset=cs.offset,
                    ap=[cs.ap[0], [0, 2], cs.ap[1], cs.ap[2]],
                )
                sin_b = bass.AP(
                    tensor=sin_sb.tensor, offset=sn.offset,
                    ap=[sn.ap[0], [0, 2], sn.ap[1], sn.ap[2]],
                )
                tmp_sb = qkpool.tile([ROT, 2, 2, S], F16, tag="rtmp")
                nc.vector.tensor_mul(out=tmp_sb, in0=rot_ps, in1=sin_b)
                pv = qk_sb[0:ROT, :, 0, :, :]  # (32, qk, row, S)
                nc.vector.tensor_mul(out=pv, in0=pv, in1=cos_b)
                nc.vector.tensor_add(out=pv, in0=pv, in1=tmp_sb)

                return {"qk": qk_sb, "v_ps": v_ps}

            def phase1tail(p, st):
                # v PSUM->SBUF on ACT, emitted after phase2's exps so the
                # softmax chain isn't delayed; ones column = denominator
                v_sb = vpool.tile([S, 2, H, HD + 1], F16)
                nc.vector.memset(v_sb[:, :, :, HD:HD + 1], 1.0)
                nc.scalar.copy(
                    out=v_sb[:, :, :, 0:HD],
                    in_=st.pop("v_ps").rearrange("p j (h c) -> p j h c", c=HD),
                )
                st["v"] = v_sb

            def phase2(r, st):
                # scores (row-tiled) + exp + attn@[v|1] + normalize.
                # Concurrent row-group matmuls sharing a PSUM bank crash the
                # exec unit, so heads are emitted in same-group pairs (which
                # the PE serializes) with the target bank alternating per
                # group pair; the resulting head permutation in memory is
                # undone on the host (MEM_HEADS perm of W_v / Wout).
                qk_sb, v_pair = st["qk"], st["v"]
                j = r % 2
                s_ps = ps_s.tile([S, 4, 4, S], F32, tag="sps")
                for h in range(H):
                    g, ec = h % 4, h // 4
                    sl = slice(32 * g, 32 * (g + 1))
                    nc.tensor.matmul(
                        s_ps[:, g, ec, :],
                        lhsT=qk_sb[sl, 1, ec, j, :],
                        rhs=qk_sb[sl, 0, ec, j, :],
                        start=True, stop=True,
                        tile_position=(32 * g, 0),
                    )
                expT_sb = epool.tile([S, H, S], F16)
                nc.scalar.activation(
                    out=expT_sb.rearrange("p h s -> p (h s)"),
                    in_=s_ps[:, :, 0:2, :],
                    func=mybir.ActivationFunctionType.Exp,
                    bias=maskb_sb[:, r:r + 1], scale=SCALE,
                )

                o_alloc = ps_ring.tile([S, 2, D], F32, tag="ring")
                o_ps = o_alloc.rearrange("p a b -> p (a b)")[:, 0:H * (HD + 1)] \
                    .rearrange("p (h c) -> p h c", c=HD + 1)
                for m in range(H):
                    nc.tensor.matmul(
                        o_ps[:, m, :],
                        lhsT=expT_sb[:, m, :],
                        rhs=v_pair[:, j, m, :],
                        start=True, stop=True,
                    )

                recip = apool.tile([S, H], F32, tag="recip")
                nc.vector.reciprocal(out=recip, in_=o_ps[:, :, HD])
                attn_sb = apool.tile([S, H, HD], F16, tag="attn")
                recip_b = bass.AP(
                    tensor=recip.tensor, offset=recip.offset,
                    ap=list(recip.ap) + [[0, HD]],
                )
                nc.vector.tensor_mul(
                    out=attn_sb, in0=o_ps[:, :, 0:HD], in1=recip_b
                )
                st[("attn", r % 2)] = attn_sb

            def phase3(p, st):
                # paired: transpose attn -> (d, tok), project, store 2 rows
                r0 = 2 * p
                t2_alloc = ps_trot.tile([128, 2, 2, 2, S], F16, tag="trot")
                t2_ps = t2_alloc[:, 0, :, :, :]
                for j in range(2):
                    attn_flat = st.pop(("attn", j)).rearrange("p h c -> p (h c)")
                    for c in range(2):
                        nc.tensor.transpose(
                            t2_ps[:, c, j, :],
                            attn_flat[:, c * 128:(c + 1) * 128], ident,
                        )
                attnT_sb = apool.tile([128, 2, 2, S], F16, tag="attnT")
                nc.vector.tensor_copy(
                    out=attnT_sb.rearrange("p c j s -> p (c j s)"),
                    in_=t2_ps.rearrange("p c j s -> p (c j s)"),
                )

                y_ps = ps_ring.tile([S, 2, D], F32, tag="ring")
                for j in range(2):
                    for c in range(2):
                        nc.tensor.matmul(
                            y_ps[:, j, :],
                            lhsT=attnT_sb[:, c, j, :],
                            rhs=wout_sb[:, c, :],
                            start=(c == 0), stop=(c == 1),
                        )
                y_sb = ypool.tile([S, 2, D], F32)
                nc.scalar.copy(
                    out=y_sb.rearrange("p j d -> p (j d)"),
                    in_=y_ps.rearrange("p j d -> p (j d)"),
                )
                nc.sync.dma_start(
                    out=y[r0:r0 + 2].rearrange("r t d -> t r d"), in_=y_sb
                )

            # software-pipelined skew over row pairs
            npairs = RPC // 2
            state = {}
            for i in range(npairs + 2):
                if i < npairs:
                    state[i] = phase1(i)
                if 0 <= i - 2 < npairs:
                    phase3(i - 2, state[i - 2])
                if 0 <= i - 1 < npairs:
                    for j in range(2):
                        phase2(2 * (i - 1) + j, state[i - 1])
                if i < npairs:
                    phase1tail(i, state[i])
                if 0 <= i - 2 < npairs:
                    del state[i - 2]

    nc.finalize()
    return nc


_NC = None


def _get_nc():
    global _NC
    if _NC is None:
        _NC = _build_bass()
    return _NC


def _host_prep(pair_act, pair_mask, ln_gamma, ln_beta, Wqkv, Wout):
    """Build the 8 per-core input maps (numpy only)."""
    pair_act = np.ascontiguousarray(pair_act, dtype=np.float32)
    ln_gamma = np.asarray(ln_gamma, dtype=np.float32)
    ln_beta = np.asarray(ln_beta, dtype=np.float32)
    Wqkv = np.asarray(Wqkv, dtype=np.float32)
    Wout = np.asarray(Wout, dtype=np.float32)

    # fold gamma/beta into the QKV projection (beta term is exactly zero for
    # the reference's beta=0, and the kernel does not apply a qkv bias)
    W_eff = (Wqkv * ln_gamma[None, :]).T  # (256, 768): qkv = xn_z @ W_eff
    bias_eff = ln_beta @ Wqkv.T
    assert np.abs(bias_eff).max() == 0.0, "nonzero LN beta not supported"

    # rotate-half matrix R (rh = R @ qvec in channel space)
    R = np.zeros((ROT, ROT), np.float32)
    for j in range(ROT // 2):
        R[2 * j, 2 * j + 1] = -1.0
        R[2 * j + 1, 2 * j] = 1.0
    W_qrot = W_eff[:, 0:ROT] @ R.T           # (256, 32)
    W_krot = W_eff[:, D:D + ROT] @ R.T       # (256, 32)

    # head permutation: memory slot m of the scores/exp/attn tensors holds
    # head MEM_HEADS[m] (bank-cycling emission order, see phase2)
    MEM_HEADS = (0, 4, 1, 5, 2, 6, 3, 7)
    hperm = np.concatenate(
        [np.arange(h * HD, (h + 1) * HD) for h in MEM_HEADS]
    )
    W_v = W_eff[:, 2 * D:3 * D][:, hperm]      # v columns in slot order
    W_all = np.concatenate(
        [W_eff[:, 0:2 * D], W_v, W_qrot, W_krot], axis=1
    )  # (256, 832)

    wqkv_h = W_all.reshape(2, 128, EW).astype(np.float16)
    wout_h = Wout.T[hperm].reshape(2, 128, D).astype(np.float16)

    # rotary tables (transposed): table[s1, c, y]
    inv_freq = 1.0 / (10000.0 ** (np.arange(0, 16, dtype=np.float32)[::2] / 16.0))
    t = np.linspace(-1.0, 1.0, S, dtype=np.float32)
    f = np.repeat(t[:, None] * inv_freq[None, :], 2, axis=-1)  # (S, 16)
    cosT = np.empty((S, ROT, S), np.float32)
    sinT = np.empty((S, ROT, S), np.float32)
    cosT[:, :16, :] = np.cos(f)[:, :, None]
    sinT[:, :16, :] = np.sin(f)[:, :, None]
    cosT[:, 16:, :] = np.cos(f).T[None, :, :]
    sinT[:, 16:, :] = np.sin(f).T[None, :, :]
    cosT = cosT.astype(np.float16)
    sinT = sinT.astype(np.float16)

    x_all = pair_act.reshape(NROWS, S, D)
    maskb_all = np.where(
        np.asarray(pair_mask, bool), np.float32(MASK_BIAS), np.float32(0.0)
    ).reshape(NROWS, S)

    in_maps = []
    for core in range(N_CORES):
        r0 = core * RPC
        rows = slice(r0, r0 + RPC)
        s1 = np.arange(r0, r0 + RPC) % S
        in_maps.append({
            "x": x_all[rows],
            "cos_t": np.ascontiguousarray(cosT[s1].transpose(1, 0, 2)),
            "sin_t": np.ascontiguousarray(sinT[s1].transpose(1, 0, 2)),
            "maskb": np.ascontiguousarray(maskb_all[rows].T),  # (S, RPC)
            "wqkv": wqkv_h,
            "wout": wout_h,
        })
    return in_maps


def kernel(pair_act, pair_mask, ln_gamma, ln_beta, Wqkv, Wout):
    in_maps = _host_prep(pair_act, pair_mask, ln_gamma, ln_beta, Wqkv, Wout)
    nc = _get_nc()
    res = run_bass_kernel_spmd(nc, in_maps, core_ids=list(range(N_CORES)))
    y = np.stack([res.results[i]["y"] for i in range(N_CORES)])
    return y.reshape(B, S, S, D).astype(np.float32)


# revision 28
# speedup vs baseline: 1.0494x; 1.0494x over previous
"""Axial (per-row) pair attention kernel for Trainium2, 8-core SPMD.

Contract: kernel(**inputs) takes the FULL unsharded inputs from
setup_inputs() and returns the FULL (2,128,128,256) float32 output.

Sharding: the (b, s1) row axis (2*128 = 256 independent attention rows) is
split evenly across 8 NeuronCores; each core runs the identical Bass program
on its 32-row slice.

v2 design (vs the repack-based baseline):
 - Scores run as row-tiled matmuls (tile_position via base-partition
   slices): head g of chunk ec lives at partitions 32g of the natural
   QKV e-chunk layout, so the (32, head, tok) repack is gone entirely.
 - rotate_half(q) is folded into the projection weights on the host
   (W_rot = W[:, :32] @ R^T), so q_rot/k_rot fall out of the same QKV
   matmul at partitions 0-31 and rotary is 3 wide DVE ops.
 - One exp activation per row ([128, 8*128], mask as per-partition bias).
 - Copy work split across DVE (qk/v/normalize/transposeouts) and ACT
   (exp, y writeback) to balance engine busy time.
"""

import numpy as np

import concourse.bass as bass
import concourse.mybir as mybir
import concourse.tile as tile
from concourse import bacc
from concourse.bass_utils import run_bass_kernel_spmd
from concourse.masks import make_identity

N_CORES = 8
B, S, D = 2, 128, 256
H, HD, ROT = 8, 32, 32
NROWS = B * S
RPC = NROWS // N_CORES  # rows per core = 32
SCALE = HD ** -0.5
LN_EPS = 1e-5
MASK_BIAS = -1e9
EW = 3 * D + 2 * ROT  # 832 projection channels (q|k|v|qrot|krot)

F32 = mybir.dt.float32
F16 = mybir.dt.float16  # matmul-input dtype (fp32 accumulate in PSUM)


def _build_bass() -> bass.Bass:
    nc = bacc.Bacc(None)

    x = nc.dram_tensor("x", [RPC, S, D], F32, kind="ExternalInput")
    cos_t = nc.dram_tensor("cos_t", [ROT, RPC, S], F16, kind="ExternalInput")
    sin_t = nc.dram_tensor("sin_t", [ROT, RPC, S], F16, kind="ExternalInput")
    maskb = nc.dram_tensor("maskb", [S, RPC], F32, kind="ExternalInput")
    wqkv = nc.dram_tensor("wqkv", [2, 128, EW], F16, kind="ExternalInput")
    wout = nc.dram_tensor("wout", [2, 128, D], F16, kind="ExternalInput")
    y = nc.dram_tensor("y", [RPC, S, D], F32, kind="ExternalOutput")

    with tile.TileContext(nc) as tc:
        with (
            tc.tile_pool(name="consts", bufs=1) as consts,
            tc.tile_pool(name="xpool", bufs=RPC) as xpool,
            tc.tile_pool(name="lnpool", bufs=4) as lnpool,
            tc.tile_pool(name="tpool", bufs=2) as tpool,
            tc.tile_pool(name="qkpool", bufs=2) as qkpool,
            tc.tile_pool(name="vpool", bufs=2) as vpool,
            tc.tile_pool(name="epool", bufs=2) as epool,
            tc.tile_pool(name="apool", bufs=4) as apool,
            tc.tile_pool(name="ypool", bufs=2) as ypool,
            tc.tile_pool(name="ps_trot", bufs=1, space="PSUM") as ps_trot,
            tc.tile_pool(name="ps_ring", bufs=2, space="PSUM") as ps_ring,
            tc.tile_pool(name="ps_v", bufs=1, space="PSUM") as ps_v,
            tc.tile_pool(name="ps_s", bufs=1, space="PSUM") as ps_s,
        ):
            # ---- constants ----
            ident = consts.tile([128, 128], F16)
            make_identity(nc, ident)
            wqkv_sb = consts.tile([128, 2, EW], F16)
            for c in range(2):
                nc.sync.dma_start(out=wqkv_sb[:, c, :], in_=wqkv[c])
            wout_sb = consts.tile([128, 2, D], F16)
            for c in range(2):
                nc.sync.dma_start(out=wout_sb[:, c, :], in_=wout[c])
            maskb_sb = consts.tile([S, RPC], F32)
            nc.sync.dma_start(out=maskb_sb, in_=maskb[:])
            eps_sb = consts.tile([128, 1], F32)
            nc.vector.memset(eps_sb, LN_EPS)
            cos_sb = consts.tile([ROT, RPC, S], F16)
            sin_sb = consts.tile([ROT, RPC, S], F16)
            nc.sync.dma_start(out=cos_sb, in_=cos_t[:])
            nc.sync.dma_start(out=sin_sb, in_=sin_t[:])

            # ---- prologue: loads + LN statistics for all rows ----
            mv_all = consts.tile([S, RPC, 2], F32)
            rstd_all = consts.tile([S, RPC], F32)
            x_tiles = []
            for p in range(RPC // 2):
                x_sb = xpool.tile([S, 2, D], F32)
                nc.sync.dma_start(
                    out=x_sb, in_=x[2 * p:2 * p + 2].rearrange("r t d -> t r d")
                )
                x_tiles.extend([x_sb[:, 0, :], x_sb[:, 1, :]])
                stats = lnpool.tile([S, 2, 6], F32, tag="stats")
                for j in range(2):
                    nc.vector.bn_stats(out=stats[:, j, :], in_=x_sb[:, j, :])
                    nc.vector.bn_aggr(
                        out=mv_all[:, 2 * p + j, :], in_=stats[:, j, :]
                    )
            # rstd for all rows in two wide instructions
            nc.scalar.activation(
                out=rstd_all, in_=mv_all[:, :, 1],
                func=mybir.ActivationFunctionType.Sqrt,
                bias=eps_sb, scale=1.0,
            )
            nc.vector.reciprocal(out=rstd_all, in_=rstd_all)

            def phase1(p):
                # LN apply, transpose, QKV+rot projection, rotary, v
                r0 = 2 * p
                xn_pair = lnpool.tile([S, 2, D], F16, tag="xn")
                for j in range(2):
                    nc.vector.tensor_scalar(
                        out=xn_pair[:, j, :], in0=x_tiles[r0 + j],
                        scalar1=mv_all[:, r0 + j, 0:1],
                        scalar2=rstd_all[:, r0 + j:r0 + j + 1],
                        op0=mybir.AluOpType.subtract, op1=mybir.AluOpType.mult,
                    )

                # ---- transpose xn -> (d-chunk, row, tok) ----
                t_alloc = ps_trot.tile([128, 2, 2, 2, S], F16, tag="trot")
                t_ps = t_alloc[:, 0, :, :, :]
                for j in range(2):
                    for c in range(2):
                        nc.tensor.transpose(
                            t_ps[:, c, j, :],
                            xn_pair[:, j, c * 128:(c + 1) * 128], ident,
                        )
                xnT_sb = tpool.tile([128, 2, 2, S], F16)
                nc.vector.tensor_copy(
                    out=xnT_sb.rearrange("p c j s -> p (c j s)"),
                    in_=t_ps.rearrange("p c j s -> p (c j s)"),
                )

                # ---- q/k projection: out (e-chunk, row, tok), through a
                # single shared PSUM bank (q then k), v/rot MMs fill the
                # PE while each half is copied out ----
                qk_sb = qkpool.tile([128, 2, 2, 2, S], F16, tag="qksb")
                rot_ps = ps_trot.tile([ROT, 2, 2, S], F32, tag="trot")
                v_ps = ps_v.tile([S, 2, D], F32, tag="vps")
                for qk in range(2):
                    half_ps = ps_ring.tile([128, 2, 2, S], F32, tag="ring")
                    for ec in range(2):
                        col = qk * D + ec * 128
                        for dc in range(2):
                            nc.tensor.matmul(
                                half_ps[:, ec, :, :],
                                lhsT=wqkv_sb[:, dc, col:col + 128],
                                rhs=xnT_sb[:, dc, :, :],
                                start=(dc == 0), stop=(dc == 1),
                            )
                    if qk == 0:
                        # rot/v matmuls keep the PE busy during the q copy
                        for rqk in range(2):
                            col = 3 * D + rqk * ROT
                            for dc in range(2):
                                nc.tensor.matmul(
                                    rot_ps[:, rqk, :, :],
                                    lhsT=wqkv_sb[:, dc, col:col + ROT],
                                    rhs=xnT_sb[:, dc, :, :],
                                    start=(dc == 0), stop=(dc == 1),
                                )
                    else:
                        for j in range(2):
                            for dc in range(2):
                                nc.tensor.matmul(
                                    v_ps[:, j, :],
                                    lhsT=xnT_sb[:, dc, j, :],
                                    rhs=wqkv_sb[:, dc, 2 * D:3 * D],
                                    start=(dc == 0), stop=(dc == 1),
                                )
                    # q half via DVE, k half via ACT (engine balance)
                    (nc.vector.tensor_copy if qk == 0 else nc.scalar.copy)(
                        out=qk_sb[:, qk].rearrange("p e j s -> p (e j s)"),
                        in_=half_ps.rearrange("p e j s -> p (e j s)"),
                    )

                # ---- rotary on head 0 (partitions 0-31 of ec 0) ----
                cs = cos_sb[:, r0:r0 + 2, :]
                sn = sin_sb[:, r0:r0 + 2, :]
                cos_b = bass.AP(
                    tensor=cos_sb.tensor, offset=cs.offset,
                    ap=[cs.ap[0], [0, 2], cs.ap[1], cs.ap[2]],
                )
                sin_b = bass.AP(
                    tensor=sin_sb.tensor, offset=sn.offset,
                    ap=[sn.ap[0], [0, 2], sn.ap[1], sn.ap[2]],
                )
                tmp_sb = qkpool.tile([ROT, 2, 2, S], F16, tag="rtmp")
                nc.vector.tensor_mul(out=tmp_sb, in0=rot_ps, in1=sin_b)
                pv = qk_sb[0:ROT, :, 0, :, :]  # (32, qk, row, S)
                nc.vector.tensor_mul(out=pv, in0=pv, in1=cos_b)
                nc.vector.tensor_add(out=pv, in0=pv, in1=tmp_sb)

                return {"qk": qk_sb, "v_ps": v_ps}

            def phase1tail(p, st):
                # v PSUM->SBUF on ACT, emitted after phase2's exps so the
                # softmax chain isn't delayed; ones column = denominator
                v_sb = vpool.tile([S, 2, H, HD + 1], F16)
                nc.vector.memset(v_sb[:, :, :, HD:HD + 1], 1.0)
                nc.vector.tensor_copy(
                    out=v_sb[:, :, :, 0:HD],
                    in_=st.pop("v_ps").rearrange("p j (h c) -> p j h c", c=HD),
                )
                st["v"] = v_sb

            def phase2(r, st):
                # scores (row-tiled) + exp + attn@[v|1] + normalize.
                # Concurrent row-group matmuls sharing a PSUM bank crash the
                # exec unit, so heads are emitted in same-group pairs (which
                # the PE serializes) with the target bank alternating per
                # group pair; the resulting head permutation in memory is
                # undone on the host (MEM_HEADS perm of W_v / Wout).
                qk_sb, v_pair = st["qk"], st["v"]
                j = r % 2
                s_ps = ps_s.tile([S, 4, 4, S], F32, tag="sps")
                for h in range(H):
                    g, ec = h % 4, h // 4
                    sl = slice(32 * g, 32 * (g + 1))
                    nc.tensor.matmul(
                        s_ps[:, g, ec, :],
                        lhsT=qk_sb[sl, 1, ec, j, :],
                        rhs=qk_sb[sl, 0, ec, j, :],
                        start=True, stop=True,
                        tile_position=(32 * g, 0),
                    )
                expT_sb = epool.tile([S, H, S], F16)
                nc.scalar.activation(
                    out=expT_sb.rearrange("p h s -> p (h s)"),
                    in_=s_ps[:, :, 0:2, :],
                    func=mybir.ActivationFunctionType.Exp,
                    bias=maskb_sb[:, r:r + 1], scale=SCALE,
                )

                o_alloc = ps_ring.tile([S, 2, D], F32, tag="ring")
                o_ps = o_alloc.rearrange("p a b -> p (a b)")[:, 0:H * (HD + 1)] \
                    .rearrange("p (h c) -> p h c", c=HD + 1)
                for m in range(H):
                    nc.tensor.matmul(
                        o_ps[:, m, :],
                        lhsT=expT_sb[:, m, :],
                        rhs=v_pair[:, j, m, :],
                        start=True, stop=True,
                    )

                recip = apool.tile([S, H], F32, tag="recip")
                nc.vector.reciprocal(out=recip, in_=o_ps[:, :, HD])
                attn_sb = apool.tile([S, H, HD], F16, tag="attn")
                recip_b = bass.AP(
                    tensor=recip.tensor, offset=recip.offset,
                    ap=list(recip.ap) + [[0, HD]],
                )
                nc.vector.tensor_mul(
                    out=attn_sb, in0=o_ps[:, :, 0:HD], in1=recip_b
                )
                st[("attn", r % 2)] = attn_sb

            def phase3(p, st):
                # paired: transpose attn -> (d, tok), project, store 2 rows
                r0 = 2 * p
                t2_alloc = ps_trot.tile([128, 2, 2, 2, S], F16, tag="trot")
                t2_ps = t2_alloc[:, 0, :, :, :]
                for j in range(2):
                    attn_flat = st.pop(("attn", j)).rearrange("p h c -> p (h c)")
                    for c in range(2):
                        nc.tensor.transpose(
                            t2_ps[:, c, j, :],
                            attn_flat[:, c * 128:(c + 1) * 128], ident,
                        )
                attnT_sb = apool.tile([128, 2, 2, S], F16, tag="attnT")
                nc.vector.tensor_copy(
                    out=attnT_sb.rearrange("p c j s -> p (c j s)"),
                    in_=t2_ps.rearrange("p c j s -> p (c j s)"),
                )

                y_ps = ps_ring.tile([S, 2, D], F32, tag="ring")
                for j in range(2):
                    for c in range(2):
                        nc.tensor.matmul(
                            y_ps[:, j, :],
                            lhsT=attnT_sb[:, c, j, :],
                            rhs=wout_sb[:, c, :],
                            start=(c == 0), stop=(c == 1),
                        )
                y_sb = ypool.tile([S, 2, D], F32)
                nc.scalar.copy(
                    out=y_sb.rearrange("p j d -> p (j d)"),
                    in_=y_ps.rearrange("p j d -> p (j d)"),
                )
                nc.sync.dma_start(
                    out=y[r0:r0 + 2].rearrange("r t d -> t r d"), in_=y_sb
                )

            # software-pipelined skew over row pairs
            npairs = RPC // 2
            state = {}
            for i in range(npairs + 2):
                if i < npairs:
                    state[i] = phase1(i)
                if 0 <= i - 2 < npairs:
                    phase3(i - 2, state[i - 2])
                # v copy sits here so it fills the DVE wait for attnv
                if i < npairs:
                    phase1tail(i, state[i])
                if 0 <= i - 1 < npairs:
                    for j in range(2):
                        phase2(2 * (i - 1) + j, state[i - 1])
                if 0 <= i - 2 < npairs:
                    del state[i - 2]

    nc.finalize()
    return nc


_NC = None


def _get_nc():
    global _NC
    if _NC is None:
        _NC = _build_bass()
    return _NC


def _host_prep(pair_act, pair_mask, ln_gamma, ln_beta, Wqkv, Wout):
    """Build the 8 per-core input maps (numpy only)."""
    pair_act = np.ascontiguousarray(pair_act, dtype=np.float32)
    ln_gamma = np.asarray(ln_gamma, dtype=np.float32)
    ln_beta = np.asarray(ln_beta, dtype=np.float32)
    Wqkv = np.asarray(Wqkv, dtype=np.float32)
    Wout = np.asarray(Wout, dtype=np.float32)

    # fold gamma/beta into the QKV projection (beta term is exactly zero for
    # the reference's beta=0, and the kernel does not apply a qkv bias)
    W_eff = (Wqkv * ln_gamma[None, :]).T  # (256, 768): qkv = xn_z @ W_eff
    bias_eff = ln_beta @ Wqkv.T
    assert np.abs(bias_eff).max() == 0.0, "nonzero LN beta not supported"

    # rotate-half matrix R (rh = R @ qvec in channel space)
    R = np.zeros((ROT, ROT), np.float32)
    for j in range(ROT // 2):
        R[2 * j, 2 * j + 1] = -1.0
        R[2 * j + 1, 2 * j] = 1.0
    W_qrot = W_eff[:, 0:ROT] @ R.T           # (256, 32)
    W_krot = W_eff[:, D:D + ROT] @ R.T       # (256, 32)

    # head permutation: memory slot m of the scores/exp/attn tensors holds
    # head MEM_HEADS[m] (bank-cycling emission order, see phase2)
    MEM_HEADS = (0, 4, 1, 5, 2, 6, 3, 7)
    hperm = np.concatenate(
        [np.arange(h * HD, (h + 1) * HD) for h in MEM_HEADS]
    )
    W_v = W_eff[:, 2 * D:3 * D][:, hperm]      # v columns in slot order
    W_all = np.concatenate(
        [W_eff[:, 0:2 * D], W_v, W_qrot, W_krot], axis=1
    )  # (256, 832)

    wqkv_h = W_all.reshape(2, 128, EW).astype(np.float16)
    wout_h = Wout.T[hperm].reshape(2, 128, D).astype(np.float16)

    # rotary tables (transposed): table[s1, c, y]
    inv_freq = 1.0 / (10000.0 ** (np.arange(0, 16, dtype=np.float32)[::2] / 16.0))
    t = np.linspace(-1.0, 1.0, S, dtype=np.float32)
    f = np.repeat(t[:, None] * inv_freq[None, :], 2, axis=-1)  # (S, 16)
    cosT = np.empty((S, ROT, S), np.float32)
    sinT = np.empty((S, ROT, S), np.float32)
    cosT[:, :16, :] = np.cos(f)[:, :, None]
    sinT[:, :16, :] = np.sin(f)[:, :, None]
    cosT[:, 16:, :] = np.cos(f).T[None, :, :]
    sinT[:, 16:, :] = np.sin(f).T[None, :, :]
    cosT = cosT.astype(np.float16)
    sinT = sinT.astype(np.float16)

    x_all = pair_act.reshape(NROWS, S, D)
    maskb_all = np.where(
        np.asarray(pair_mask, bool), np.float32(MASK_BIAS), np.float32(0.0)
    ).reshape(NROWS, S)

    in_maps = []
    for core in range(N_CORES):
        r0 = core * RPC
        rows = slice(r0, r0 + RPC)
        s1 = np.arange(r0, r0 + RPC) % S
        in_maps.append({
            "x": x_all[rows],
            "cos_t": np.ascontiguousarray(cosT[s1].transpose(1, 0, 2)),
            "sin_t": np.ascontiguousarray(sinT[s1].transpose(1, 0, 2)),
            "maskb": np.ascontiguousarray(maskb_all[rows].T),  # (S, RPC)
            "wqkv": wqkv_h,
            "wout": wout_h,
        })
    return in_maps


def kernel(pair_act, pair_mask, ln_gamma, ln_beta, Wqkv, Wout):
    in_maps = _host_prep(pair_act, pair_mask, ln_gamma, ln_beta, Wqkv, Wout)
    nc = _get_nc()
    res = run_bass_kernel_spmd(nc, in_maps, core_ids=list(range(N_CORES)))
    y = np.stack([res.results[i]["y"] for i in range(N_CORES)])
    return y.reshape(B, S, S, D).astype(np.float32)


# revision 29
# speedup vs baseline: 1.0538x; 1.0042x over previous
"""Axial (per-row) pair attention kernel for Trainium2, 8-core SPMD.

Contract: kernel(**inputs) takes the FULL unsharded inputs from
setup_inputs() and returns the FULL (2,128,128,256) float32 output.

Sharding: the (b, s1) row axis (2*128 = 256 independent attention rows) is
split evenly across 8 NeuronCores; each core runs the identical Bass program
on its 32-row slice.

v2 design (vs the repack-based baseline):
 - Scores run as row-tiled matmuls (tile_position via base-partition
   slices): head g of chunk ec lives at partitions 32g of the natural
   QKV e-chunk layout, so the (32, head, tok) repack is gone entirely.
 - rotate_half(q) is folded into the projection weights on the host
   (W_rot = W[:, :32] @ R^T), so q_rot/k_rot fall out of the same QKV
   matmul at partitions 0-31 and rotary is 3 wide DVE ops.
 - One exp activation per row ([128, 8*128], mask as per-partition bias).
 - Copy work split across DVE (qk/v/normalize/transposeouts) and ACT
   (exp, y writeback) to balance engine busy time.
"""

import numpy as np

import concourse.bass as bass
import concourse.mybir as mybir
import concourse.tile as tile
from concourse import bacc
from concourse.bass_utils import run_bass_kernel_spmd
from concourse.masks import make_identity

N_CORES = 8
B, S, D = 2, 128, 256
H, HD, ROT = 8, 32, 32
NROWS = B * S
RPC = NROWS // N_CORES  # rows per core = 32
SCALE = HD ** -0.5
LN_EPS = 1e-5
MASK_BIAS = -1e9
EW = 3 * D + 2 * ROT  # 832 projection channels (q|k|v|qrot|krot)

F32 = mybir.dt.float32
F16 = mybir.dt.float16  # matmul-input dtype (fp32 accumulate in PSUM)


def _build_bass() -> bass.Bass:
    nc = bacc.Bacc(None)

    x = nc.dram_tensor("x", [RPC, S, D], F32, kind="ExternalInput")
    cos_t = nc.dram_tensor("cos_t", [ROT, RPC, S], F16, kind="ExternalInput")
    sin_t = nc.dram_tensor("sin_t", [ROT, RPC, S], F16, kind="ExternalInput")
    maskb = nc.dram_tensor("maskb", [S, RPC], F32, kind="ExternalInput")
    wqkv = nc.dram_tensor("wqkv", [2, 128, EW], F16, kind="ExternalInput")
    wout = nc.dram_tensor("wout", [2, 128, D], F16, kind="ExternalInput")
    y = nc.dram_tensor("y", [RPC, S, D], F32, kind="ExternalOutput")

    with tile.TileContext(nc) as tc:
        with (
            tc.tile_pool(name="consts", bufs=1) as consts,
            tc.tile_pool(name="xpool", bufs=RPC) as xpool,
            tc.tile_pool(name="lnpool", bufs=4) as lnpool,
            tc.tile_pool(name="tpool", bufs=2) as tpool,
            tc.tile_pool(name="qkpool", bufs=2) as qkpool,
            tc.tile_pool(name="vpool", bufs=2) as vpool,
            tc.tile_pool(name="epool", bufs=2) as epool,
            tc.tile_pool(name="apool", bufs=4) as apool,
            tc.tile_pool(name="ypool", bufs=2) as ypool,
            tc.tile_pool(name="ps_trot", bufs=1, space="PSUM") as ps_trot,
            tc.tile_pool(name="ps_ring", bufs=2, space="PSUM") as ps_ring,
            tc.tile_pool(name="ps_v", bufs=1, space="PSUM") as ps_v,
            tc.tile_pool(name="ps_s", bufs=1, space="PSUM") as ps_s,
        ):
            # ---- constants ----
            ident = consts.tile([128, 128], F16)
            make_identity(nc, ident)
            wqkv_sb = consts.tile([128, 2, EW], F16)
            for c in range(2):
                nc.sync.dma_start(out=wqkv_sb[:, c, :], in_=wqkv[c])
            wout_sb = consts.tile([128, 2, D], F16)
            for c in range(2):
                nc.sync.dma_start(out=wout_sb[:, c, :], in_=wout[c])
            eps_sb = consts.tile([128, 1], F32)
            nc.vector.memset(eps_sb, LN_EPS)

            # ---- prologue: loads + LN statistics for all rows.
            # x DMAs go first; the remaining constant tables (cos/sin/mask,
            # ~1MB) are queued behind the first x pairs so bn_stats starts
            # as early as possible. ----
            mv_all = consts.tile([S, RPC, 2], F32)
            rstd_all = consts.tile([S, RPC], F32)
            maskb_sb = consts.tile([S, RPC], F32)
            cos_sb = consts.tile([ROT, RPC, S], F16)
            sin_sb = consts.tile([ROT, RPC, S], F16)
            x_tiles = []
            for p in range(RPC // 2):
                x_sb = xpool.tile([S, 2, D], F32)
                nc.sync.dma_start(
                    out=x_sb, in_=x[2 * p:2 * p + 2].rearrange("r t d -> t r d")
                )
                if p == 2:
                    nc.sync.dma_start(out=maskb_sb, in_=maskb[:])
                    nc.sync.dma_start(out=cos_sb, in_=cos_t[:])
                    nc.sync.dma_start(out=sin_sb, in_=sin_t[:])
                x_tiles.extend([x_sb[:, 0, :], x_sb[:, 1, :]])
                stats = lnpool.tile([S, 2, 6], F32, tag="stats")
                for j in range(2):
                    nc.vector.bn_stats(out=stats[:, j, :], in_=x_sb[:, j, :])
                    nc.vector.bn_aggr(
                        out=mv_all[:, 2 * p + j, :], in_=stats[:, j, :]
                    )
            # rstd for all rows in two wide instructions
            nc.scalar.activation(
                out=rstd_all, in_=mv_all[:, :, 1],
                func=mybir.ActivationFunctionType.Sqrt,
                bias=eps_sb, scale=1.0,
            )
            nc.vector.reciprocal(out=rstd_all, in_=rstd_all)

            def phase1(p):
                # LN apply, transpose, QKV+rot projection, rotary, v
                r0 = 2 * p
                xn_pair = lnpool.tile([S, 2, D], F16, tag="xn")
                for j in range(2):
                    nc.vector.tensor_scalar(
                        out=xn_pair[:, j, :], in0=x_tiles[r0 + j],
                        scalar1=mv_all[:, r0 + j, 0:1],
                        scalar2=rstd_all[:, r0 + j:r0 + j + 1],
                        op0=mybir.AluOpType.subtract, op1=mybir.AluOpType.mult,
                    )

                # ---- transpose xn -> (d-chunk, row, tok) ----
                t_alloc = ps_trot.tile([128, 2, 2, 2, S], F16, tag="trot")
                t_ps = t_alloc[:, 0, :, :, :]
                for j in range(2):
                    for c in range(2):
                        nc.tensor.transpose(
                            t_ps[:, c, j, :],
                            xn_pair[:, j, c * 128:(c + 1) * 128], ident,
                        )
                xnT_sb = tpool.tile([128, 2, 2, S], F16)
                nc.vector.tensor_copy(
                    out=xnT_sb.rearrange("p c j s -> p (c j s)"),
                    in_=t_ps.rearrange("p c j s -> p (c j s)"),
                )

                # ---- q/k projection: out (e-chunk, row, tok), through a
                # single shared PSUM bank (q then k), v/rot MMs fill the
                # PE while each half is copied out ----
                qk_sb = qkpool.tile([128, 2, 2, 2, S], F16, tag="qksb")
                rot_ps = ps_trot.tile([ROT, 2, 2, S], F32, tag="trot")
                v_ps = ps_v.tile([S, 2, D], F32, tag="vps")
                for qk in range(2):
                    half_ps = ps_ring.tile([128, 2, 2, S], F32, tag="ring")
                    for ec in range(2):
                        col = qk * D + ec * 128
                        for dc in range(2):
                            nc.tensor.matmul(
                                half_ps[:, ec, :, :],
                                lhsT=wqkv_sb[:, dc, col:col + 128],
                                rhs=xnT_sb[:, dc, :, :],
                                start=(dc == 0), stop=(dc == 1),
                            )
                    if qk == 0:
                        # rot/v matmuls keep the PE busy during the q copy
                        for rqk in range(2):
                            col = 3 * D + rqk * ROT
                            for dc in range(2):
                                nc.tensor.matmul(
                                    rot_ps[:, rqk, :, :],
                                    lhsT=wqkv_sb[:, dc, col:col + ROT],
                                    rhs=xnT_sb[:, dc, :, :],
                                    start=(dc == 0), stop=(dc == 1),
                                )
                    else:
                        for j in range(2):
                            for dc in range(2):
                                nc.tensor.matmul(
                                    v_ps[:, j, :],
                                    lhsT=xnT_sb[:, dc, j, :],
                                    rhs=wqkv_sb[:, dc, 2 * D:3 * D],
                                    start=(dc == 0), stop=(dc == 1),
                                )
                    # q half via DVE, k half via ACT (engine balance)
                    (nc.vector.tensor_copy if qk == 0 else nc.scalar.copy)(
                        out=qk_sb[:, qk].rearrange("p e j s -> p (e j s)"),
                        in_=half_ps.rearrange("p e j s -> p (e j s)"),
                    )

                # ---- rotary on head 0 (partitions 0-31 of ec 0) ----
                cs = cos_sb[:, r0:r0 + 2, :]
                sn = sin_sb[:, r0:r0 + 2, :]
                cos_b = bass.AP(
                    tensor=cos_sb.tensor, offset=cs.offset,
                    ap=[cs.ap[0], [0, 2], cs.ap[1], cs.ap[2]],
                )
                sin_b = bass.AP(
                    tensor=sin_sb.tensor, offset=sn.offset,
                    ap=[sn.ap[0], [0, 2], sn.ap[1], sn.ap[2]],
                )
                tmp_sb = qkpool.tile([ROT, 2, 2, S], F16, tag="rtmp")
                nc.vector.tensor_mul(out=tmp_sb, in0=rot_ps, in1=sin_b)
                pv = qk_sb[0:ROT, :, 0, :, :]  # (32, qk, row, S)
                nc.vector.tensor_mul(out=pv, in0=pv, in1=cos_b)
                nc.vector.tensor_add(out=pv, in0=pv, in1=tmp_sb)

                return {"qk": qk_sb, "v_ps": v_ps}

            def phase1tail(p, st):
                # v PSUM->SBUF on ACT, emitted after phase2's exps so the
                # softmax chain isn't delayed; ones column = denominator
                v_sb = vpool.tile([S, 2, H, HD + 1], F16)
                nc.vector.memset(v_sb[:, :, :, HD:HD + 1], 1.0)
                nc.vector.tensor_copy(
                    out=v_sb[:, :, :, 0:HD],
                    in_=st.pop("v_ps").rearrange("p j (h c) -> p j h c", c=HD),
                )
                st["v"] = v_sb

            def phase2(r, st):
                # scores (row-tiled) + exp + attn@[v|1] + normalize.
                # Concurrent row-group matmuls sharing a PSUM bank crash the
                # exec unit, so heads are emitted in same-group pairs (which
                # the PE serializes) with the target bank alternating per
                # group pair; the resulting head permutation in memory is
                # undone on the host (MEM_HEADS perm of W_v / Wout).
                qk_sb, v_pair = st["qk"], st["v"]
                j = r % 2
                s_ps = ps_s.tile([S, 4, 4, S], F32, tag="sps")
                for h in range(H):
                    g, ec = h % 4, h // 4
                    sl = slice(32 * g, 32 * (g + 1))
                    nc.tensor.matmul(
                        s_ps[:, g, ec, :],
                        lhsT=qk_sb[sl, 1, ec, j, :],
                        rhs=qk_sb[sl, 0, ec, j, :],
                        start=True, stop=True,
                        tile_position=(32 * g, 0),
                    )
                expT_sb = epool.tile([S, H, S], F16)
                nc.scalar.activation(
                    out=expT_sb.rearrange("p h s -> p (h s)"),
                    in_=s_ps[:, :, 0:2, :],
                    func=mybir.ActivationFunctionType.Exp,
                    bias=maskb_sb[:, r:r + 1], scale=SCALE,
                )

                o_alloc = ps_ring.tile([S, 2, D], F32, tag="ring")
                o_ps = o_alloc.rearrange("p a b -> p (a b)")[:, 0:H * (HD + 1)] \
                    .rearrange("p (h c) -> p h c", c=HD + 1)
                for m in range(H):
                    nc.tensor.matmul(
                        o_ps[:, m, :],
                        lhsT=expT_sb[:, m, :],
                        rhs=v_pair[:, j, m, :],
                        start=True, stop=True,
                    )

                recip = apool.tile([S, H], F32, tag="recip")
                nc.vector.reciprocal(out=recip, in_=o_ps[:, :, HD])
                attn_sb = apool.tile([S, H, HD], F16, tag="attn")
                recip_b = bass.AP(
                    tensor=recip.tensor, offset=recip.offset,
                    ap=list(recip.ap) + [[0, HD]],
                )
                nc.vector.tensor_mul(
                    out=attn_sb, in0=o_ps[:, :, 0:HD], in1=recip_b
                )
                st[("attn", r % 2)] = attn_sb

            def phase3(p, st):
                # paired: transpose attn -> (d, tok), project, store 2 rows
                r0 = 2 * p
                t2_alloc = ps_trot.tile([128, 2, 2, 2, S], F16, tag="trot")
                t2_ps = t2_alloc[:, 0, :, :, :]
                for j in range(2):
                    attn_flat = st.pop(("attn", j)).rearrange("p h c -> p (h c)")
                    for c in range(2):
                        nc.tensor.transpose(
                            t2_ps[:, c, j, :],
                            attn_flat[:, c * 128:(c + 1) * 128], ident,
                        )
                attnT_sb = apool.tile([128, 2, 2, S], F16, tag="attnT")
                nc.vector.tensor_copy(
                    out=attnT_sb.rearrange("p c j s -> p (c j s)"),
                    in_=t2_ps.rearrange("p c j s -> p (c j s)"),
                )

                y_ps = ps_ring.tile([S, 2, D], F32, tag="ring")
                for j in range(2):
                    for c in range(2):
                        nc.tensor.matmul(
                            y_ps[:, j, :],
                            lhsT=attnT_sb[:, c, j, :],
                            rhs=wout_sb[:, c, :],
                            start=(c == 0), stop=(c == 1),
                        )
                y_sb = ypool.tile([S, 2, D], F32)
                nc.scalar.copy(
                    out=y_sb.rearrange("p j d -> p (j d)"),
                    in_=y_ps.rearrange("p j d -> p (j d)"),
                )
                nc.sync.dma_start(
                    out=y[r0:r0 + 2].rearrange("r t d -> t r d"), in_=y_sb
                )

            # software-pipelined skew over row pairs
            npairs = RPC // 2
            state = {}
            for i in range(npairs + 2):
                if i < npairs:
                    state[i] = phase1(i)
                if 0 <= i - 2 < npairs:
                    phase3(i - 2, state[i - 2])
                # v copy sits here so it fills the DVE wait for attnv
                if i < npairs:
                    phase1tail(i, state[i])
                if 0 <= i - 1 < npairs:
                    for j in range(2):
                        phase2(2 * (i - 1) + j, state[i - 1])
                if 0 <= i - 2 < npairs:
                    del state[i - 2]

    nc.finalize()
    return nc


_NC = None


def _get_nc():
    global _NC
    if _NC is None:
        _NC = _build_bass()
    return _NC


def _host_prep(pair_act, pair_mask, ln_gamma, ln_beta, Wqkv, Wout):
    """Build the 8 per-core input maps (numpy only)."""
    pair_act = np.ascontiguousarray(pair_act, dtype=np.float32)
    ln_gamma = np.asarray(ln_gamma, dtype=np.float32)
    ln_beta = np.asarray(ln_beta, dtype=np.float32)
    Wqkv = np.asarray(Wqkv, dtype=np.float32)
    Wout = np.asarray(Wout, dtype=np.float32)

    # fold gamma/beta into the QKV projection (beta term is exactly zero for
    # the reference's beta=0, and the kernel does not apply a qkv bias)
    W_eff = (Wqkv * ln_gamma[None, :]).T  # (256, 768): qkv = xn_z @ W_eff
    bias_eff = ln_beta @ Wqkv.T
    assert np.abs(bias_eff).max() == 0.0, "nonzero LN beta not supported"

    # rotate-half matrix R (rh = R @ qvec in channel space)
    R = np.zeros((ROT, ROT), np.float32)
    for j in range(ROT // 2):
        R[2 * j, 2 * j + 1] = -1.0
        R[2 * j + 1, 2 * j] = 1.0
    W_qrot = W_eff[:, 0:ROT] @ R.T           # (256, 32)
    W_krot = W_eff[:, D:D + ROT] @ R.T       # (256, 32)

    # head permutation: memory slot m of the scores/exp/attn tensors holds
    # head MEM_HEADS[m] (bank-cycling emission order, see phase2)
    MEM_HEADS = (0, 4, 1, 5, 2, 6, 3, 7)
    hperm = np.concatenate(
        [np.arange(h * HD, (h + 1) * HD) for h in MEM_HEADS]
    )
    W_v = W_eff[:, 2 * D:3 * D][:, hperm]      # v columns in slot order
    W_all = np.concatenate(
        [W_eff[:, 0:2 * D], W_v, W_qrot, W_krot], axis=1
    )  # (256, 832)

    wqkv_h = W_all.reshape(2, 128, EW).astype(np.float16)
    wout_h = Wout.T[hperm].reshape(2, 128, D).astype(np.float16)

    # rotary tables (transposed): table[s1, c, y]
    inv_freq = 1.0 / (10000.0 ** (np.arange(0, 16, dtype=np.float32)[::2] / 16.0))
    t = np.linspace(-1.0, 1.0, S, dtype=np.float32)
    f = np.repeat(t[:, None] * inv_freq[None, :], 2, axis=-1)  # (S, 16)
    cosT = np.empty((S, ROT, S), np.float32)
    sinT = np.empty((S, ROT, S), np.float32)
    cosT[:, :16, :] = np.cos(f)[:, :, None]
    sinT[:, :16, :] = np.sin(f)[:, :, None]
    cosT[:, 16:, :] = np.cos(f).T[None, :, :]
    sinT[:, 16:, :] = np.sin(f).T[None, :, :]
    cosT = cosT.astype(np.float16)
    sinT = sinT.astype(np.float16)

    x_all = pair_act.reshape(NROWS, S, D)
    maskb_all = np.where(
        np.asarray(pair_mask, bool), np.float32(MASK_BIAS), np.float32(0.0)
    ).reshape(NROWS, S)

    in_maps = []
    for core in range(N_CORES):
        r0 = core * RPC
        rows = slice(r0, r0 + RPC)
        s1 = np.arange(r0, r0 + RPC) % S
        in_maps.append({
            "x": x_all[rows],
            "cos_t": np.ascontiguousarray(cosT[s1].transpose(1, 0, 2)),
            "sin_t": np.ascontiguousarray(sinT[s1].transpose(1, 0, 2)),
            "maskb": np.ascontiguousarray(maskb_all[rows].T),  # (S, RPC)
            "wqkv": wqkv_h,
            "wout": wout_h,
        })
    return in_maps


def kernel(pair_act, pair_mask, ln_gamma, ln_beta, Wqkv, Wout):
    in_maps = _host_prep(pair_act, pair_mask, ln_gamma, ln_beta, Wqkv, Wout)
    nc = _get_nc()
    res = run_bass_kernel_spmd(nc, in_maps, core_ids=list(range(N_CORES)))
    y = np.stack([res.results[i]["y"] for i in range(N_CORES)])
    return y.reshape(B, S, S, D).astype(np.float32)


# revision 30
# speedup vs baseline: 1.1117x; 1.0550x over previous
"""Axial (per-row) pair attention kernel for Trainium2, 8-core SPMD.

Contract: kernel(**inputs) takes the FULL unsharded inputs from
setup_inputs() and returns the FULL (2,128,128,256) float32 output.

Sharding: the (b, s1) row axis (2*128 = 256 independent attention rows) is
split evenly across 8 NeuronCores; each core runs the identical Bass program
on its 32-row slice.

v2 design (vs the repack-based baseline):
 - Scores run as row-tiled matmuls (tile_position via base-partition
   slices): head g of chunk ec lives at partitions 32g of the natural
   QKV e-chunk layout, so the (32, head, tok) repack is gone entirely.
 - rotate_half(q) is folded into the projection weights on the host
   (W_rot = W[:, :32] @ R^T), so q_rot/k_rot fall out of the same QKV
   matmul at partitions 0-31 and rotary is 3 wide DVE ops.
 - One exp activation per row ([128, 8*128], mask as per-partition bias).
 - Copy work split across DVE (qk/v/normalize/transposeouts) and ACT
   (exp, y writeback) to balance engine busy time.
"""

import numpy as np

import concourse.bass as bass
import concourse.mybir as mybir
import concourse.tile as tile
from concourse import bacc
from concourse.bass_utils import run_bass_kernel_spmd
from concourse.masks import make_identity

N_CORES = 8
B, S, D = 2, 128, 256
H, HD, ROT = 8, 32, 32
NROWS = B * S
RPC = NROWS // N_CORES  # rows per core = 32
SCALE = HD ** -0.5
LN_EPS = 1e-5
MASK_BIAS = -1e9
EW = 3 * D + 2 * ROT  # 832 projection channels (q|k|v|qrot|krot)

F32 = mybir.dt.float32
F16 = mybir.dt.float16  # matmul-input dtype (fp32 accumulate in PSUM)


def _build_bass() -> bass.Bass:
    nc = bacc.Bacc(None)

    x = nc.dram_tensor("x", [RPC, S, D], F32, kind="ExternalInput")
    cos_t = nc.dram_tensor("cos_t", [ROT, RPC, S], F16, kind="ExternalInput")
    sin_t = nc.dram_tensor("sin_t", [ROT, RPC, S], F16, kind="ExternalInput")
    maskb = nc.dram_tensor("maskb", [S, RPC], F32, kind="ExternalInput")
    wqkv = nc.dram_tensor("wqkv", [2, 128, EW], F16, kind="ExternalInput")
    wout = nc.dram_tensor("wout", [2, 128, D], F16, kind="ExternalInput")
    y = nc.dram_tensor("y", [RPC, S, D], F32, kind="ExternalOutput")

    with tile.TileContext(nc) as tc:
        with (
            tc.tile_pool(name="consts", bufs=1) as consts,
            tc.tile_pool(name="xpool", bufs=RPC) as xpool,
            tc.tile_pool(name="lnpool", bufs=4) as lnpool,
            tc.tile_pool(name="tpool", bufs=2) as tpool,
            tc.tile_pool(name="qkpool", bufs=2) as qkpool,
            tc.tile_pool(name="vpool", bufs=2) as vpool,
            tc.tile_pool(name="epool", bufs=2) as epool,
            tc.tile_pool(name="apool", bufs=4) as apool,
            tc.tile_pool(name="ypool", bufs=2) as ypool,
            tc.tile_pool(name="ps_trot", bufs=1, space="PSUM") as ps_trot,
            tc.tile_pool(name="ps_ring", bufs=2, space="PSUM") as ps_ring,
            tc.tile_pool(name="ps_v", bufs=1, space="PSUM") as ps_v,
            tc.tile_pool(name="ps_s", bufs=1, space="PSUM") as ps_s,
        ):
            # ---- constants ----
            ident = consts.tile([128, 128], F16)
            make_identity(nc, ident)
            wqkv_sb = consts.tile([128, 2, EW], F16)
            for c in range(2):
                nc.sync.dma_start(out=wqkv_sb[:, c, :], in_=wqkv[c])
            wout_sb = consts.tile([128, 2, D], F16)
            for c in range(2):
                nc.sync.dma_start(out=wout_sb[:, c, :], in_=wout[c])
            eps_sb = consts.tile([128, 1], F32)
            nc.vector.memset(eps_sb, LN_EPS)

            # ---- prologue: loads + LN statistics for all rows.
            # x DMAs go first; the remaining constant tables (cos/sin/mask,
            # ~1MB) are queued behind the first x pairs so bn_stats starts
            # as early as possible. ----
            mv_all = consts.tile([S, RPC, 2], F32)
            rstd_all = consts.tile([S, RPC], F32)
            maskb_sb = consts.tile([S, RPC], F32)
            cos_sb = consts.tile([ROT, RPC, S], F16)
            sin_sb = consts.tile([ROT, RPC, S], F16)
            x_tiles = []
            for p in range(RPC // 2):
                x_sb = xpool.tile([S, 2, D], F32)
                nc.sync.dma_start(
                    out=x_sb, in_=x[2 * p:2 * p + 2].rearrange("r t d -> t r d")
                )
                x_tiles.extend([x_sb[:, 0, :], x_sb[:, 1, :]])
                stats = lnpool.tile([S, 2, 6], F32, tag="stats")
                for j in range(2):
                    nc.vector.bn_stats(out=stats[:, j, :], in_=x_sb[:, j, :])
                    nc.vector.bn_aggr(
                        out=mv_all[:, 2 * p + j, :], in_=stats[:, j, :]
                    )
            nc.sync.dma_start(out=maskb_sb, in_=maskb[:])
            nc.sync.dma_start(out=cos_sb, in_=cos_t[:])
            nc.sync.dma_start(out=sin_sb, in_=sin_t[:])
            # rstd for all rows in two wide instructions
            nc.scalar.activation(
                out=rstd_all, in_=mv_all[:, :, 1],
                func=mybir.ActivationFunctionType.Sqrt,
                bias=eps_sb, scale=1.0,
            )
            nc.vector.reciprocal(out=rstd_all, in_=rstd_all)

            def phase1(p):
                # LN apply, transpose, QKV+rot projection, rotary, v
                r0 = 2 * p
                xn_pair = lnpool.tile([S, 2, D], F16, tag="xn")
                for j in range(2):
                    nc.vector.tensor_scalar(
                        out=xn_pair[:, j, :], in0=x_tiles[r0 + j],
                        scalar1=mv_all[:, r0 + j, 0:1],
                        scalar2=rstd_all[:, r0 + j:r0 + j + 1],
                        op0=mybir.AluOpType.subtract, op1=mybir.AluOpType.mult,
                    )

                # ---- transpose xn -> (d-chunk, row, tok) ----
                t_alloc = ps_trot.tile([128, 2, 2, 2, S], F16, tag="trot")
                t_ps = t_alloc[:, 0, :, :, :]
                for j in range(2):
                    for c in range(2):
                        nc.tensor.transpose(
                            t_ps[:, c, j, :],
                            xn_pair[:, j, c * 128:(c + 1) * 128], ident,
                        )
                xnT_sb = tpool.tile([128, 2, 2, S], F16)
                nc.vector.tensor_copy(
                    out=xnT_sb.rearrange("p c j s -> p (c j s)"),
                    in_=t_ps.rearrange("p c j s -> p (c j s)"),
                )

                # ---- q/k projection: out (e-chunk, row, tok), through a
                # single shared PSUM bank (q then k), v/rot MMs fill the
                # PE while each half is copied out ----
                qk_sb = qkpool.tile([128, 2, 2, 2, S], F16, tag="qksb")
                rot_ps = ps_trot.tile([ROT, 2, 2, S], F32, tag="trot")
                v_ps = ps_v.tile([S, 2, D], F32, tag="vps")
                for qk in range(2):
                    half_ps = ps_ring.tile([128, 2, 2, S], F32, tag="ring")
                    for ec in range(2):
                        col = qk * D + ec * 128
                        for dc in range(2):
                            nc.tensor.matmul(
                                half_ps[:, ec, :, :],
                                lhsT=wqkv_sb[:, dc, col:col + 128],
                                rhs=xnT_sb[:, dc, :, :],
                                start=(dc == 0), stop=(dc == 1),
                            )
                    if qk == 0:
                        # rot/v matmuls keep the PE busy during the q copy
                        for rqk in range(2):
                            col = 3 * D + rqk * ROT
                            for dc in range(2):
                                nc.tensor.matmul(
                                    rot_ps[:, rqk, :, :],
                                    lhsT=wqkv_sb[:, dc, col:col + ROT],
                                    rhs=xnT_sb[:, dc, :, :],
                                    start=(dc == 0), stop=(dc == 1),
                                )
                    else:
                        for j in range(2):
                            for dc in range(2):
                                nc.tensor.matmul(
                                    v_ps[:, j, :],
                                    lhsT=xnT_sb[:, dc, j, :],
                                    rhs=wqkv_sb[:, dc, 2 * D:3 * D],
                                    start=(dc == 0), stop=(dc == 1),
                                )
                    # q half via DVE, k half via ACT (engine balance)
                    (nc.vector.tensor_copy if qk == 0 else nc.scalar.copy)(
                        out=qk_sb[:, qk].rearrange("p e j s -> p (e j s)"),
                        in_=half_ps.rearrange("p e j s -> p (e j s)"),
                    )

                # ---- rotary on head 0 (partitions 0-31 of ec 0) ----
                cs = cos_sb[:, r0:r0 + 2, :]
                sn = sin_sb[:, r0:r0 + 2, :]
                cos_b = bass.AP(
                    tensor=cos_sb.tensor, offset=cs.offset,
                    ap=[cs.ap[0], [0, 2], cs.ap[1], cs.ap[2]],
                )
                sin_b = bass.AP(
                    tensor=sin_sb.tensor, offset=sn.offset,
                    ap=[sn.ap[0], [0, 2], sn.ap[1], sn.ap[2]],
                )
                tmp_sb = qkpool.tile([ROT, 2, 2, S], F16, tag="rtmp")
                nc.vector.tensor_mul(out=tmp_sb, in0=rot_ps, in1=sin_b)
                pv = qk_sb[0:ROT, :, 0, :, :]  # (32, qk, row, S)
                nc.vector.tensor_mul(out=pv, in0=pv, in1=cos_b)
                nc.vector.tensor_add(out=pv, in0=pv, in1=tmp_sb)

                return {"qk": qk_sb, "v_ps": v_ps}

            def phase1tail(p, st):
                # v PSUM->SBUF on ACT, emitted after phase2's exps so the
                # softmax chain isn't delayed; ones column = denominator
                v_sb = vpool.tile([S, 2, H, HD + 1], F16)
                nc.vector.memset(v_sb[:, :, :, HD:HD + 1], 1.0)
                nc.vector.tensor_copy(
                    out=v_sb[:, :, :, 0:HD],
                    in_=st.pop("v_ps").rearrange("p j (h c) -> p j h c", c=HD),
                )
                st["v"] = v_sb

            def phase2(r, st):
                # scores (row-tiled) + exp + attn@[v|1] + normalize.
                # Concurrent row-group matmuls sharing a PSUM bank crash the
                # exec unit, so heads are emitted in same-group pairs (which
                # the PE serializes) with the target bank alternating per
                # group pair; the resulting head permutation in memory is
                # undone on the host (MEM_HEADS perm of W_v / Wout).
                qk_sb, v_pair = st["qk"], st["v"]
                j = r % 2
                s_ps = ps_s.tile([S, 4, 4, S], F32, tag="sps")
                for h in range(H):
                    g, ec = h % 4, h // 4
                    sl = slice(32 * g, 32 * (g + 1))
                    nc.tensor.matmul(
                        s_ps[:, g, ec, :],
                        lhsT=qk_sb[sl, 1, ec, j, :],
                        rhs=qk_sb[sl, 0, ec, j, :],
                        start=True, stop=True,
                        tile_position=(32 * g, 0),
                    )
                expT_sb = epool.tile([S, H, S], F16)
                nc.scalar.activation(
                    out=expT_sb.rearrange("p h s -> p (h s)"),
                    in_=s_ps[:, :, 0:2, :],
                    func=mybir.ActivationFunctionType.Exp,
                    bias=maskb_sb[:, r:r + 1], scale=SCALE,
                )

                o_alloc = ps_ring.tile([S, 2, D], F32, tag="ring")
                o_ps = o_alloc.rearrange("p a b -> p (a b)")[:, 0:H * (HD + 1)] \
                    .rearrange("p (h c) -> p h c", c=HD + 1)
                for m in range(H):
                    nc.tensor.matmul(
                        o_ps[:, m, :],
                        lhsT=expT_sb[:, m, :],
                        rhs=v_pair[:, j, m, :],
                        start=True, stop=True,
                    )

                recip = apool.tile([S, H], F32, tag="recip")
                nc.vector.reciprocal(out=recip, in_=o_ps[:, :, HD])
                attn_sb = apool.tile([S, H, HD], F16, tag="attn")
                recip_b = bass.AP(
                    tensor=recip.tensor, offset=recip.offset,
                    ap=list(recip.ap) + [[0, HD]],
                )
                nc.vector.tensor_mul(
                    out=attn_sb, in0=o_ps[:, :, 0:HD], in1=recip_b
                )
                st[("attn", r % 2)] = attn_sb

            def phase3(p, st):
                # paired: transpose attn -> (d, tok), project, store 2 rows
                r0 = 2 * p
                t2_alloc = ps_trot.tile([128, 2, 2, 2, S], F16, tag="trot")
                t2_ps = t2_alloc[:, 0, :, :, :]
                for j in range(2):
                    attn_flat = st.pop(("attn", j)).rearrange("p h c -> p (h c)")
                    for c in range(2):
                        nc.tensor.transpose(
                            t2_ps[:, c, j, :],
                            attn_flat[:, c * 128:(c + 1) * 128], ident,
                        )
                attnT_sb = apool.tile([128, 2, 2, S], F16, tag="attnT")
                nc.vector.tensor_copy(
                    out=attnT_sb.rearrange("p c j s -> p (c j s)"),
                    in_=t2_ps.rearrange("p c j s -> p (c j s)"),
                )

                y_ps = ps_ring.tile([S, 2, D], F32, tag="ring")
                for j in range(2):
                    for c in range(2):
                        nc.tensor.matmul(
                            y_ps[:, j, :],
                            lhsT=attnT_sb[:, c, j, :],
                            rhs=wout_sb[:, c, :],
                            start=(c == 0), stop=(c == 1),
                        )
                y_sb = ypool.tile([S, 2, D], F32)
                nc.scalar.copy(
                    out=y_sb.rearrange("p j d -> p (j d)"),
                    in_=y_ps.rearrange("p j d -> p (j d)"),
                )
                nc.sync.dma_start(
                    out=y[r0:r0 + 2].rearrange("r t d -> t r d"), in_=y_sb
                )

            # software-pipelined skew over row pairs
            npairs = RPC // 2
            state = {}
            for i in range(npairs + 2):
                if i < npairs:
                    state[i] = phase1(i)
                if 0 <= i - 2 < npairs:
                    phase3(i - 2, state[i - 2])
                # v copy sits here so it fills the DVE wait for attnv
                if i < npairs:
                    phase1tail(i, state[i])
                if 0 <= i - 1 < npairs:
                    for j in range(2):
                        phase2(2 * (i - 1) + j, state[i - 1])
                if 0 <= i - 2 < npairs:
                    del state[i - 2]

    nc.finalize()
    return nc


_NC = None


def _get_nc():
    global _NC
    if _NC is None:
        _NC = _build_bass()
    return _NC


def _host_prep(pair_act, pair_mask, ln_gamma, ln_beta, Wqkv, Wout):
    """Build the 8 per-core input maps (numpy only)."""
    pair_act = np.ascontiguousarray(pair_act, dtype=np.float32)
    ln_gamma = np.asarray(ln_gamma, dtype=np.float32)
    ln_beta = np.asarray(ln_beta, dtype=np.float32)
    Wqkv = np.asarray(Wqkv, dtype=np.float32)
    Wout = np.asarray(Wout, dtype=np.float32)

    # fold gamma/beta into the QKV projection (beta term is exactly zero for
    # the reference's beta=0, and the kernel does not apply a qkv bias)
    W_eff = (Wqkv * ln_gamma[None, :]).T  # (256, 768): qkv = xn_z @ W_eff
    bias_eff = ln_beta @ Wqkv.T
    assert np.abs(bias_eff).max() == 0.0, "nonzero LN beta not supported"

    # rotate-half matrix R (rh = R @ qvec in channel space)
    R = np.zeros((ROT, ROT), np.float32)
    for j in range(ROT // 2):
        R[2 * j, 2 * j + 1] = -1.0
        R[2 * j + 1, 2 * j] = 1.0
    W_qrot = W_eff[:, 0:ROT] @ R.T           # (256, 32)
    W_krot = W_eff[:, D:D + ROT] @ R.T       # (256, 32)

    # head permutation: memory slot m of the scores/exp/attn tensors holds
    # head MEM_HEADS[m] (bank-cycling emission order, see phase2)
    MEM_HEADS = (0, 4, 1, 5, 2, 6, 3, 7)
    hperm = np.concatenate(
        [np.arange(h * HD, (h + 1) * HD) for h in MEM_HEADS]
    )
    W_v = W_eff[:, 2 * D:3 * D][:, hperm]      # v columns in slot order
    W_all = np.concatenate(
        [W_eff[:, 0:2 * D], W_v, W_qrot, W_krot], axis=1
    )  # (256, 832)

    wqkv_h = W_all.reshape(2, 128, EW).astype(np.float16)
    wout_h = Wout.T[hperm].reshape(2, 128, D).astype(np.float16)

    # rotary tables (transposed): table[s1, c, y]
    inv_freq = 1.0 / (10000.0 ** (np.arange(0, 16, dtype=np.float32)[::2] / 16.0))
    t = np.linspace(-1.0, 1.0, S, dtype=np.float32)
    f = np.repeat(t[:, None] * inv_freq[None, :], 2, axis=-1)  # (S, 16)
    cosT = np.empty((S, ROT, S), np.float32)
    sinT = np.empty((S, ROT, S), np.float32)
    cosT[:, :16, :] = np.cos(f)[:, :, None]
    sinT[:, :16, :] = np.sin(f)[:, :, None]
    cosT[:, 16:, :] = np.cos(f).T[None, :, :]
    sinT[:, 16:, :] = np.sin(f).T[None, :, :]
    cosT = cosT.astype(np.float16)
    sinT = sinT.astype(np.float16)

    x_all = pair_act.reshape(NROWS, S, D)
    maskb_all = np.where(
        np.asarray(pair_mask, bool), np.float32(MASK_BIAS), np.float32(0.0)
    ).reshape(NROWS, S)

    in_maps = []
    for core in range(N_CORES):
        r0 = core * RPC
        rows = slice(r0, r0 + RPC)
        s1 = np.arange(r0, r0 + RPC) % S
        in_maps.append({
            "x": x_all[rows],
            "cos_t": np.ascontiguousarray(cosT[s1].transpose(1, 0, 2)),
            "sin_t": np.ascontiguousarray(sinT[s1].transpose(1, 0, 2)),
            "maskb": np.ascontiguousarray(maskb_all[rows].T),  # (S, RPC)
            "wqkv": wqkv_h,
            "wout": wout_h,
        })
    return in_maps


def kernel(pair_act, pair_mask, ln_gamma, ln_beta, Wqkv, Wout):
    in_maps = _host_prep(pair_act, pair_mask, ln_gamma, ln_beta, Wqkv, Wout)
    nc = _get_nc()
    res = run_bass_kernel_spmd(nc, in_maps, core_ids=list(range(N_CORES)))
    y = np.stack([res.results[i]["y"] for i in range(N_CORES)])
    return y.reshape(B, S, S, D).astype(np.float32)


# revision 31
# speedup vs baseline: 1.1182x; 1.0059x over previous
"""Axial (per-row) pair attention kernel for Trainium2, 8-core SPMD.

Contract: kernel(**inputs) takes the FULL unsharded inputs from
setup_inputs() and returns the FULL (2,128,128,256) float32 output.

Sharding: the (b, s1) row axis (2*128 = 256 independent attention rows) is
split evenly across 8 NeuronCores; each core runs the identical Bass program
on its 32-row slice.

v2 design (vs the repack-based baseline):
 - Scores run as row-tiled matmuls (tile_position via base-partition
   slices): head g of chunk ec lives at partitions 32g of the natural
   QKV e-chunk layout, so the (32, head, tok) repack is gone entirely.
 - rotate_half(q) is folded into the projection weights on the host
   (W_rot = W[:, :32] @ R^T), so q_rot/k_rot fall out of the same QKV
   matmul at partitions 0-31 and rotary is 3 wide DVE ops.
 - One exp activation per row ([128, 8*128], mask as per-partition bias).
 - Copy work split across DVE (qk/v/normalize/transposeouts) and ACT
   (exp, y writeback) to balance engine busy time.
"""

import numpy as np

import concourse.bass as bass
import concourse.mybir as mybir
import concourse.tile as tile
from concourse import bacc
from concourse.bass_utils import run_bass_kernel_spmd
from concourse.masks import make_identity

N_CORES = 8
B, S, D = 2, 128, 256
H, HD, ROT = 8, 32, 32
NROWS = B * S
RPC = NROWS // N_CORES  # rows per core = 32
SCALE = HD ** -0.5
LN_EPS = 1e-5
MASK_BIAS = -1e9
EW = 3 * D + 2 * ROT  # 832 projection channels (q|k|v|qrot|krot)

F32 = mybir.dt.float32
F16 = mybir.dt.float16  # matmul-input dtype (fp32 accumulate in PSUM)


def _build_bass() -> bass.Bass:
    nc = bacc.Bacc(None)

    x = nc.dram_tensor("x", [RPC, S, D], F32, kind="ExternalInput")
    cos_t = nc.dram_tensor("cos_t", [ROT, RPC, S], F16, kind="ExternalInput")
    sin_t = nc.dram_tensor("sin_t", [ROT, RPC, S], F16, kind="ExternalInput")
    maskb = nc.dram_tensor("maskb", [S, RPC], F32, kind="ExternalInput")
    wqkv = nc.dram_tensor("wqkv", [2, 128, EW], F16, kind="ExternalInput")
    wout = nc.dram_tensor("wout", [2, 128, D], F16, kind="ExternalInput")
    y = nc.dram_tensor("y", [RPC, S, D], F32, kind="ExternalOutput")

    with tile.TileContext(nc) as tc:
        with (
            tc.tile_pool(name="consts", bufs=1) as consts,
            tc.tile_pool(name="xpool", bufs=RPC) as xpool,
            tc.tile_pool(name="lnpool", bufs=4) as lnpool,
            tc.tile_pool(name="tpool", bufs=2) as tpool,
            tc.tile_pool(name="qkpool", bufs=2) as qkpool,
            tc.tile_pool(name="vpool", bufs=2) as vpool,
            tc.tile_pool(name="epool", bufs=2) as epool,
            tc.tile_pool(name="apool", bufs=4) as apool,
            tc.tile_pool(name="ypool", bufs=2) as ypool,
            tc.tile_pool(name="ps_trot", bufs=1, space="PSUM") as ps_trot,
            tc.tile_pool(name="ps_ring", bufs=2, space="PSUM") as ps_ring,
            tc.tile_pool(name="ps_v", bufs=1, space="PSUM") as ps_v,
            tc.tile_pool(name="ps_s", bufs=1, space="PSUM") as ps_s,
        ):
            # ---- constants ----
            ident = consts.tile([128, 128], F16)
            make_identity(nc, ident)
            wqkv_sb = consts.tile([128, 2, EW], F16)
            for c in range(2):
                nc.sync.dma_start(out=wqkv_sb[:, c, :], in_=wqkv[c])
            wout_sb = consts.tile([128, 2, D], F16)
            for c in range(2):
                nc.sync.dma_start(out=wout_sb[:, c, :], in_=wout[c])
            eps_sb = consts.tile([128, 1], F32)
            nc.vector.memset(eps_sb, LN_EPS)

            # ---- prologue: loads + LN statistics for all rows.
            # x DMAs go first; the remaining constant tables (cos/sin/mask,
            # ~1MB) are queued behind the first x pairs so bn_stats starts
            # as early as possible. ----
            mv_all = consts.tile([S, RPC, 2], F32)
            rstd_all = consts.tile([S, RPC], F32)
            maskb_sb = consts.tile([S, RPC], F32)
            cos_sb = consts.tile([ROT, RPC, S], F16)
            sin_sb = consts.tile([ROT, RPC, S], F16)
            x_tiles = []
            for p in range(RPC // 2):
                x_sb = xpool.tile([S, 2, D], F32)
                nc.sync.dma_start(
                    out=x_sb, in_=x[2 * p:2 * p + 2].rearrange("r t d -> t r d")
                )
                x_tiles.extend([x_sb[:, 0, :], x_sb[:, 1, :]])
                stats = lnpool.tile([S, 2, 6], F32, tag="stats")
                for j in range(2):
                    nc.vector.bn_stats(out=stats[:, j, :], in_=x_sb[:, j, :])
                    nc.vector.bn_aggr(
                        out=mv_all[:, 2 * p + j, :], in_=stats[:, j, :]
                    )
            nc.sync.dma_start(out=maskb_sb, in_=maskb[:])
            nc.sync.dma_start(out=cos_sb, in_=cos_t[:])
            nc.sync.dma_start(out=sin_sb, in_=sin_t[:])
            # rstd for all rows in two wide instructions
            nc.scalar.activation(
                out=rstd_all, in_=mv_all[:, :, 1],
                func=mybir.ActivationFunctionType.Sqrt,
                bias=eps_sb, scale=1.0,
            )
            nc.vector.reciprocal(out=rstd_all, in_=rstd_all)

            def phase1(p):
                # LN apply, transpose, QKV+rot projection, rotary, v
                r0 = 2 * p
                xn_pair = lnpool.tile([S, 2, D], F16, tag="xn")
                for j in range(2):
                    nc.vector.tensor_scalar(
                        out=xn_pair[:, j, :], in0=x_tiles[r0 + j],
                        scalar1=mv_all[:, r0 + j, 0:1],
                        scalar2=rstd_all[:, r0 + j:r0 + j + 1],
                        op0=mybir.AluOpType.subtract, op1=mybir.AluOpType.mult,
                    )

                # ---- transpose xn -> (d-chunk, row, tok) ----
                t_alloc = ps_trot.tile([128, 2, 2, 2, S], F16, tag="trot")
                t_ps = t_alloc[:, 0, :, :, :]
                for j in range(2):
                    for c in range(2):
                        nc.tensor.transpose(
                            t_ps[:, c, j, :],
                            xn_pair[:, j, c * 128:(c + 1) * 128], ident,
                        )
                xnT_sb = tpool.tile([128, 2, 2, S], F16)
                nc.vector.tensor_copy(
                    out=xnT_sb.rearrange("p c j s -> p (c j s)"),
                    in_=t_ps.rearrange("p c j s -> p (c j s)"),
                )

                # ---- q/k projection: out (e-chunk, row, tok), through a
                # single shared PSUM bank (q then k), v/rot MMs fill the
                # PE while each half is copied out ----
                qk_sb = qkpool.tile([128, 2, 2, 2, S], F16, tag="qksb")
                rot_ps = ps_trot.tile([ROT, 2, 2, S], F32, tag="trot")
                v_ps = ps_v.tile([S, 2, D], F32, tag="vps")
                for qk in range(2):
                    half_ps = ps_ring.tile([128, 2, 2, S], F32, tag="ring")
                    for ec in range(2):
                        col = qk * D + ec * 128
                        for dc in range(2):
                            nc.tensor.matmul(
                                half_ps[:, ec, :, :],
                                lhsT=wqkv_sb[:, dc, col:col + 128],
                                rhs=xnT_sb[:, dc, :, :],
                                start=(dc == 0), stop=(dc == 1),
                            )
                    if qk == 0:
                        # rot/v matmuls keep the PE busy during the q copy
                        for rqk in range(2):
                            col = 3 * D + rqk * ROT
                            for dc in range(2):
                                nc.tensor.matmul(
                                    rot_ps[:, rqk, :, :],
                                    lhsT=wqkv_sb[:, dc, col:col + ROT],
                                    rhs=xnT_sb[:, dc, :, :],
                                    start=(dc == 0), stop=(dc == 1),
                                )
                    else:
                        for j in range(2):
                            for dc in range(2):
                                nc.tensor.matmul(
                                    v_ps[:, j, :],
                                    lhsT=xnT_sb[:, dc, j, :],
                                    rhs=wqkv_sb[:, dc, 2 * D:3 * D],
                                    start=(dc == 0), stop=(dc == 1),
                                )
                    # q half via DVE, k half via ACT (engine balance)
                    (nc.vector.tensor_copy if qk == 0 else nc.scalar.copy)(
                        out=qk_sb[:, qk].rearrange("p e j s -> p (e j s)"),
                        in_=half_ps.rearrange("p e j s -> p (e j s)"),
                    )

                # ---- rotary on head 0 (partitions 0-31 of ec 0) ----
                cs = cos_sb[:, r0:r0 + 2, :]
                sn = sin_sb[:, r0:r0 + 2, :]
                cos_b = bass.AP(
                    tensor=cos_sb.tensor, offset=cs.offset,
                    ap=[cs.ap[0], [0, 2], cs.ap[1], cs.ap[2]],
                )
                sin_b = bass.AP(
                    tensor=sin_sb.tensor, offset=sn.offset,
                    ap=[sn.ap[0], [0, 2], sn.ap[1], sn.ap[2]],
                )
                tmp_sb = qkpool.tile([ROT, 2, 2, S], F16, tag="rtmp")
                nc.vector.tensor_mul(out=tmp_sb, in0=rot_ps, in1=sin_b)
                pv = qk_sb[0:ROT, :, 0, :, :]  # (32, qk, row, S)
                nc.vector.tensor_mul(out=pv, in0=pv, in1=cos_b)
                nc.vector.tensor_add(out=pv, in0=pv, in1=tmp_sb)

                return {"qk": qk_sb, "v_ps": v_ps}

            def phase1tail(p, st):
                # v PSUM->SBUF on ACT, emitted after phase2's exps so the
                # softmax chain isn't delayed; ones column = denominator
                v_sb = vpool.tile([S, 2, H, HD + 1], F16)
                nc.vector.memset(v_sb[:, :, :, HD:HD + 1], 1.0)
                nc.vector.tensor_copy(
                    out=v_sb[:, :, :, 0:HD],
                    in_=st.pop("v_ps").rearrange("p j (h c) -> p j h c", c=HD),
                )
                st["v"] = v_sb

            def phase2(r, st):
                # scores (row-tiled) + exp + attn@[v|1] + normalize.
                # Concurrent row-group matmuls sharing a PSUM bank crash the
                # exec unit, so heads are emitted in same-group pairs (which
                # the PE serializes) with the target bank alternating per
                # group pair; the resulting head permutation in memory is
                # undone on the host (MEM_HEADS perm of W_v / Wout).
                qk_sb, v_pair = st["qk"], st["v"]
                j = r % 2
                s_ps = ps_s.tile([S, 4, 4, S], F32, tag="sps")
                for h in range(H):
                    g, ec = h % 4, h // 4
                    sl = slice(32 * g, 32 * (g + 1))
                    nc.tensor.matmul(
                        s_ps[:, g, ec, :],
                        lhsT=qk_sb[sl, 1, ec, j, :],
                        rhs=qk_sb[sl, 0, ec, j, :],
                        start=True, stop=True,
                        tile_position=(32 * g, 0),
                    )
                expT_sb = epool.tile([S, H, S], F16)
                nc.scalar.activation(
                    out=expT_sb.rearrange("p h s -> p (h s)"),
                    in_=s_ps[:, :, 0:2, :],
                    func=mybir.ActivationFunctionType.Exp,
                    bias=maskb_sb[:, r:r + 1], scale=SCALE,
                )

                o_alloc = ps_ring.tile([S, 2, D], F32, tag="ring")
                o_ps = o_alloc.rearrange("p a b -> p (a b)")[:, 0:H * (HD + 1)] \
                    .rearrange("p (h c) -> p h c", c=HD + 1)
                for m in range(H):
                    nc.tensor.matmul(
                        o_ps[:, m, :],
                        lhsT=expT_sb[:, m, :],
                        rhs=v_pair[:, j, m, :],
                        start=True, stop=True,
                    )

                recip = apool.tile([S, H], F32, tag="recip")
                nc.vector.reciprocal_approx_fast(out=recip, in_=o_ps[:, :, HD])
                attn_sb = apool.tile([S, H, HD], F16, tag="attn")
                recip_b = bass.AP(
                    tensor=recip.tensor, offset=recip.offset,
                    ap=list(recip.ap) + [[0, HD]],
                )
                nc.vector.tensor_mul(
                    out=attn_sb, in0=o_ps[:, :, 0:HD], in1=recip_b
                )
                st[("attn", r % 2)] = attn_sb

            def phase3(p, st):
                # paired: transpose attn -> (d, tok), project, store 2 rows
                r0 = 2 * p
                t2_alloc = ps_trot.tile([128, 2, 2, 2, S], F16, tag="trot")
                t2_ps = t2_alloc[:, 0, :, :, :]
                for j in range(2):
                    attn_flat = st.pop(("attn", j)).rearrange("p h c -> p (h c)")
                    for c in range(2):
                        nc.tensor.transpose(
                            t2_ps[:, c, j, :],
                            attn_flat[:, c * 128:(c + 1) * 128], ident,
                        )
                attnT_sb = apool.tile([128, 2, 2, S], F16, tag="attnT")
                nc.vector.tensor_copy(
                    out=attnT_sb.rearrange("p c j s -> p (c j s)"),
                    in_=t2_ps.rearrange("p c j s -> p (c j s)"),
                )

                y_ps = ps_ring.tile([S, 2, D], F32, tag="ring")
                for j in range(2):
                    for c in range(2):
                        nc.tensor.matmul(
                            y_ps[:, j, :],
                            lhsT=attnT_sb[:, c, j, :],
                            rhs=wout_sb[:, c, :],
                            start=(c == 0), stop=(c == 1),
                        )
                y_sb = ypool.tile([S, 2, D], F32)
                nc.scalar.copy(
                    out=y_sb.rearrange("p j d -> p (j d)"),
                    in_=y_ps.rearrange("p j d -> p (j d)"),
                )
                nc.sync.dma_start(
                    out=y[r0:r0 + 2].rearrange("r t d -> t r d"), in_=y_sb
                )

            # software-pipelined skew over row pairs
            npairs = RPC // 2
            state = {}
            for i in range(npairs + 2):
                if i < npairs:
                    state[i] = phase1(i)
                if 0 <= i - 2 < npairs:
                    phase3(i - 2, state[i - 2])
                # v copy sits here so it fills the DVE wait for attnv
                if i < npairs:
                    phase1tail(i, state[i])
                if 0 <= i - 1 < npairs:
                    for j in range(2):
                        phase2(2 * (i - 1) + j, state[i - 1])
                if 0 <= i - 2 < npairs:
                    del state[i - 2]

    nc.finalize()
    return nc


_NC = None


def _get_nc():
    global _NC
    if _NC is None:
        _NC = _build_bass()
    return _NC


def _host_prep(pair_act, pair_mask, ln_gamma, ln_beta, Wqkv, Wout):
    """Build the 8 per-core input maps (numpy only)."""
    pair_act = np.ascontiguousarray(pair_act, dtype=np.float32)
    ln_gamma = np.asarray(ln_gamma, dtype=np.float32)
    ln_beta = np.asarray(ln_beta, dtype=np.float32)
    Wqkv = np.asarray(Wqkv, dtype=np.float32)
    Wout = np.asarray(Wout, dtype=np.float32)

    # fold gamma/beta into the QKV projection (beta term is exactly zero for
    # the reference's beta=0, and the kernel does not apply a qkv bias)
    W_eff = (Wqkv * ln_gamma[None, :]).T  # (256, 768): qkv = xn_z @ W_eff
    bias_eff = ln_beta @ Wqkv.T
    assert np.abs(bias_eff).max() == 0.0, "nonzero LN beta not supported"

    # rotate-half matrix R (rh = R @ qvec in channel space)
    R = np.zeros((ROT, ROT), np.float32)
    for j in range(ROT // 2):
        R[2 * j, 2 * j + 1] = -1.0
        R[2 * j + 1, 2 * j] = 1.0
    W_qrot = W_eff[:, 0:ROT] @ R.T           # (256, 32)
    W_krot = W_eff[:, D:D + ROT] @ R.T       # (256, 32)

    # head permutation: memory slot m of the scores/exp/attn tensors holds
    # head MEM_HEADS[m] (bank-cycling emission order, see phase2)
    MEM_HEADS = (0, 4, 1, 5, 2, 6, 3, 7)
    hperm = np.concatenate(
        [np.arange(h * HD, (h + 1) * HD) for h in MEM_HEADS]
    )
    W_v = W_eff[:, 2 * D:3 * D][:, hperm]      # v columns in slot order
    W_all = np.concatenate(
        [W_eff[:, 0:2 * D], W_v, W_qrot, W_krot], axis=1
    )  # (256, 832)

    wqkv_h = W_all.reshape(2, 128, EW).astype(np.float16)
    wout_h = Wout.T[hperm].reshape(2, 128, D).astype(np.float16)

    # rotary tables (transposed): table[s1, c, y]
    inv_freq = 1.0 / (10000.0 ** (np.arange(0, 16, dtype=np.float32)[::2] / 16.0))
    t = np.linspace(-1.0, 1.0, S, dtype=np.float32)
    f = np.repeat(t[:, None] * inv_freq[None, :], 2, axis=-1)  # (S, 16)
    cosT = np.empty((S, ROT, S), np.float32)
    sinT = np.empty((S, ROT, S), np.float32)
    cosT[:, :16, :] = np.cos(f)[:, :, None]
    sinT[:, :16, :] = np.sin(f)[:, :, None]
    cosT[:, 16:, :] = np.cos(f).T[None, :, :]
    sinT[:, 16:, :] = np.sin(f).T[None, :, :]
    cosT = cosT.astype(np.float16)
    sinT = sinT.astype(np.float16)

    x_all = pair_act.reshape(NROWS, S, D)
    maskb_all = np.where(
        np.asarray(pair_mask, bool), np.float32(MASK_BIAS), np.float32(0.0)
    ).reshape(NROWS, S)

    in_maps = []
    for core in range(N_CORES):
        r0 = core * RPC
        rows = slice(r0, r0 + RPC)
        s1 = np.arange(r0, r0 + RPC) % S
        in_maps.append({
            "x": x_all[rows],
            "cos_t": np.ascontiguousarray(cosT[s1].transpose(1, 0, 2)),
            "sin_t": np.ascontiguousarray(sinT[s1].transpose(1, 0, 2)),
            "maskb": np.ascontiguousarray(maskb_all[rows].T),  # (S, RPC)
            "wqkv": wqkv_h,
            "wout": wout_h,
        })
    return in_maps


def kernel(pair_act, pair_mask, ln_gamma, ln_beta, Wqkv, Wout):
    in_maps = _host_prep(pair_act, pair_mask, ln_gamma, ln_beta, Wqkv, Wout)
    nc = _get_nc()
    res = run_bass_kernel_spmd(nc, in_maps, core_ids=list(range(N_CORES)))
    y = np.stack([res.results[i]["y"] for i in range(N_CORES)])
    return y.reshape(B, S, S, D).astype(np.float32)
